# revision 9
# baseline (speedup 1.0000x reference)
"""Trainium2 Bass kernel for nn_BitwiseHashing.

Computes out = tanh(mean_l(x) @ W.T + b) for x:[12,8192,1024] f32,
W:[64,1024], b:[64] -> out:[8192,64].

Strategy (data-parallel over 8 NeuronCores):
  - shard x along batch dim: 1024 rows per core (48 MiB each, streamed).
  - host pre-transposes W to wt = (W.T / L) [1024,64]; bias shipped as [1,64].
  - per 128-row block: stream 12 L-slices (contiguous 512 KiB DMAs),
    accumulate with DVE adds, PE-transpose the 8 [128,128] d-chunks of the
    sum, matmul against wt chunks accumulating in PSUM [128,64] (bias
    pre-loaded via a C=1 ones-matmul), tanh on ScalarE, DMA out [128,64].
"""

import numpy as np

import concourse.bacc as bacc
import concourse.mybir as mybir
from concourse import tile
from concourse.masks import make_identity
from concourse.bass_utils import run_bass_kernel_spmd

L, B, D, K = 12, 8192, 1024, 64
NCORES = 8
BS = B // NCORES      # 1024 batch rows per core
P = 128               # partitions
NBLK = BS // P        # 8 row blocks per core
NDC = D // P          # 8 contraction chunks
F32 = mybir.dt.float32

_nc_cache = None


def _build():
    global _nc_cache
    if _nc_cache is not None:
        return _nc_cache

    nc = bacc.Bacc("TRN2", target_bir_lowering=False, debug=False)
    x = nc.dram_tensor("x", [L, BS, D], F32, kind="ExternalInput")
    wt = nc.dram_tensor("wt", [D, K], F32, kind="ExternalInput")
    bias = nc.dram_tensor("bias", [1, K], F32, kind="ExternalInput")
    y = nc.dram_tensor("y", [BS, K], F32, kind="ExternalOutput")

    with tile.TileContext(nc) as tc:
        with (
            tc.tile_pool(name="const", bufs=1) as cpool,
            tc.tile_pool(name="xin", bufs=18) as xpool,
            tc.tile_pool(name="xt", bufs=12) as tpool,
            tc.tile_pool(name="out", bufs=3) as opool,
            tc.tile_pool(name="pt", bufs=4, space="PSUM") as pt_pool,
            tc.tile_pool(name="po", bufs=2, space="PSUM") as po_pool,
        ):
            # constants go over the SWDGE queue to keep both HWDGE rings
            # free for the x stream from t=0
            wt_sb = cpool.tile([P, NDC * K], F32)
            for dc in range(NDC):
                nc.gpsimd.dma_start(
                    out=wt_sb[:, dc * K:(dc + 1) * K],
                    in_=wt.ap()[dc * P:(dc + 1) * P, :],
                )
            bias_sb = cpool.tile([1, K], F32)
            nc.gpsimd.dma_start(out=bias_sb[:], in_=bias.ap())
            ones_sb = cpool.tile([1, P], F32)
            nc.gpsimd.memset(ones_sb[:], 1.0)
            ident = cpool.tile([P, P], F32)
            make_identity(nc, ident[:])

            xap = x.ap()
            yap = y.ap()

            def issue_loads(blk):
                b0 = blk * P
                xt = []
                for l in range(L):
                    xl = xpool.tile([P, D], F32)
                    eng = nc.sync if l % 2 == 0 else nc.scalar
                    eng.dma_start(out=xl[:], in_=xap[l, b0:b0 + P, :])
                    xt.append(xl)
                return xt

            def compute(blk, xt):
                b0 = blk * P
                # running sum of the first 10 slices (hidden under the
                # stream); the last pair reduces separately so only 2 adds
                # remain on the critical path after the final tile arrives
                acc = xt[0]
                nc.vector.tensor_add(out=acc[:], in0=acc[:], in1=xt[1][:])
                for l in range(2, L - 2):
                    nc.vector.tensor_add(out=acc[:], in0=acc[:], in1=xt[l][:])
                tail2 = xt[L - 2]
                nc.vector.tensor_add(
                    out=tail2[:], in0=tail2[:], in1=xt[L - 1][:]
                )
                nc.vector.tensor_add(out=acc[:], in0=acc[:], in1=tail2[:])

                po = po_pool.tile([P, K], F32)
                # bias broadcast across partitions: ones[1,128].T @ bias[1,64]
                nc.tensor.matmul(
                    po[:], lhsT=ones_sb[:], rhs=bias_sb[:], start=True, stop=False
                )
                for dc in range(NDC):
                    pt = pt_pool.tile([P, P], F32)
                    nc.tensor.transpose(
                        pt[:], acc[:, dc * P:(dc + 1) * P], ident[:]
                    )
                    xT = tpool.tile([P, P], F32)
                    nc.scalar.copy(out=xT[:], in_=pt[:])
                    nc.tensor.matmul(
                        po[:],
                        lhsT=xT[:],
                        rhs=wt_sb[:, dc * K:(dc + 1) * K],
                        start=False,
                        stop=(dc == NDC - 1),
                    )
                ot = opool.tile([P, K], F32)
                nc.scalar.activation(
                    ot[:], po[:], mybir.ActivationFunctionType.Tanh
                )
                # y is tiny — keep it off the two x-stream rings
                nc.gpsimd.dma_start(out=yap[b0:b0 + P, :], in_=ot[:])

            # software-pipelined emission: block n+1's DMA triggers are
            # emitted before block n's compute so the per-engine instruction
            # streams never stall the x rings behind compute
            pending = issue_loads(0)
            for blk in range(NBLK):
                nxt = issue_loads(blk + 1) if blk + 1 < NBLK else None
                compute(blk, pending)
                pending = nxt

    nc.compile()
    _nc_cache = nc
    return nc


def _ensure_ntff_hook():
    """Register the axon NTFF profile hook if the image's antenv lacks it."""
    import sys
    import types

    try:
        from antenv.axon_hooks import get_axon_ntff_profile_hook  # noqa: F401
        return
    except ImportError:
        pass
    import antenv

    mod = types.ModuleType("antenv.axon_hooks")
    mod._hook = None

    def set_axon_ntff_profile_hook(h):
        mod._hook = h

    def get_axon_ntff_profile_hook():
        return mod._hook

    mod.set_axon_ntff_profile_hook = set_axon_ntff_profile_hook
    mod.get_axon_ntff_profile_hook = get_axon_ntff_profile_hook
    sys.modules["antenv.axon_hooks"] = mod
    antenv.axon_hooks = mod
    try:
        from trn_agent_boot.trn_boot import _ntff_profile_via_ctypes

        mod._hook = _ntff_profile_via_ctypes("/opt/axon/libaxon_pjrt.so")
    except Exception:
        mod._hook = None


def _run(inputs, trace=False, **kwargs):
    x = np.asarray(inputs["x"], dtype=np.float32)
    W = np.asarray(inputs["W"], dtype=np.float32)
    b = np.asarray(inputs["b"], dtype=np.float32)
    wt = np.ascontiguousarray(W.T).astype(np.float32) * np.float32(1.0 / L)
    bias = np.ascontiguousarray(b.reshape(1, K)).astype(np.float32)
    in_maps = [
        {
            "x": np.ascontiguousarray(x[:, c * BS:(c + 1) * BS, :]),
            "wt": wt,
            "bias": bias,
        }
        for c in range(NCORES)
    ]
    if trace:
        _ensure_ntff_hook()
        import concourse.bass_utils as bu

        bu.upload_artifacts = lambda tmpdir: "local://skipped"
    nc = _build()
    res = run_bass_kernel_spmd(
        nc, in_maps, core_ids=list(range(NCORES)), trace=trace, **kwargs
    )
    y = np.concatenate([r["y"] for r in res.results], axis=0)
    return y, res


def kernel(**inputs):
    y, _ = _run(inputs)
    return y
